# revision 11
# baseline (speedup 1.0000x reference)
"""Trainium2 Bass kernel for nn_DCTLayer: per-8x8-block 2D DCT-like transform.

Math: reference computes, per 8x8 block X of the 256x256 image,
    out_block[y, v] = sum_x A[v, x] * X[x, y],   where A = D @ D
(D = 8x8 DCT basis). out_block = (A @ X)^T.

Kernel strategy (per core, pure data parallel over batch):
  - Load 2 images per DMA, casting f32 -> fp16 in the SWDGE DMA (halves
    HBM read bytes). SBUF layout: partition = row (32Gh + 8Gl + x),
    free = (img, r, col) with col = 32Jh + 8Jl + y.
  - ONE matmul per image with the constant 128x128 block-diagonal BD (16
    copies of A^T, fp16) as the STATIONARY operand and the image as the
    512-wide fp16 moving operand (1 cycle/row):
        z[32Gh+8Gl+v, f=(r, 32Jh+8Jl+y)] = sum_x A[v,x] X[row(G,x), col]
  - The required within-8x8-block transpose swaps v (partition-low) with
    y (free-low). Two DVE StreamTranspose passes (32x32 logical block
    transposes) with bit-field relabeling folded into the access
    patterns perform it entirely on-chip:
      ST1: logical f view (r,y | Jh,Jl): swaps p-low5 (Gl,v) <-> (Jh,Jl)
           -> z1[p = 32Gh+4Jh+Jl, f-phys = 256r + 32v+8Gl+y]
      ST2: identity views: swaps (Jh,Jl) <-> (Gl,y)
           -> z2[p = 32Gh+8Gl+y = out row, f-phys = 256r + 32Jh+8Jl+v]
  - Store z2 contiguously in f32 (1KB runs per row).
"""

import sys

sys.path.insert(0, "/opt/trn_rl_repo")

from contextlib import ExitStack

import numpy as np

import concourse.bass as bass  # noqa: F401
import concourse.tile as tile
from concourse import bacc, mybir
from concourse.bass_utils import run_bass_kernel_spmd

P = 8
H = W = 256
B, C = 16, 64
NCORES = 8
BPC = B // NCORES  # batches per core
IMGS = BPC * C  # images (b,c planes) per core
ROWS = IMGS * H  # dram rows per core

TRACE = False
LAST_RESULTS = None

_nc_cache = None


def _ensure_ntff_hook():
    """The agent image's antenv lacks axon_hooks; synthesize it so
    run_bass_kernel_spmd(trace=True) can capture NTFF profiles."""
    import types

    if "antenv.axon_hooks" in sys.modules:
        return
    try:
        sys.path.insert(0, "/root/.axon_site/trn_agent_boot")
        from trn_boot import _ntff_profile_via_ctypes

        hook = _ntff_profile_via_ctypes("/opt/axon/libaxon_pjrt.so")
    except Exception:
        hook = None
    mod = types.ModuleType("antenv.axon_hooks")
    mod._hook = hook
    mod.get_axon_ntff_profile_hook = lambda: mod._hook
    mod.set_axon_ntff_profile_hook = lambda h: setattr(mod, "_hook", h)
    sys.modules["antenv.axon_hooks"] = mod


def _dct_kernel(tc, o, x, bd):
    nc = tc.nc
    f32 = mybir.dt.float32
    f16 = mybir.dt.float16
    with ExitStack() as ctx:
        xpool = ctx.enter_context(tc.tile_pool(name="xin", bufs=3))
        zcpool = ctx.enter_context(tc.tile_pool(name="zc", bufs=4))
        z1pool = ctx.enter_context(tc.tile_pool(name="z1", bufs=4))
        z2pool = ctx.enter_context(tc.tile_pool(name="z2", bufs=4))
        z3pool = ctx.enter_context(tc.tile_pool(name="z3", bufs=4))
        cpool = ctx.enter_context(tc.tile_pool(name="const", bufs=1))
        ppool = ctx.enter_context(tc.tile_pool(name="ps", bufs=8, space="PSUM"))

        bdt = cpool.tile([128, 128], f16)
        nc.gpsimd.dma_start(bdt[:], bd[:])

        for pair in range(IMGS // 2):
            # ---- load 2 fp16 images as [p=row%128, (i r c)], 512B runs ----
            xt = xpool.tile([128, 4 * W], f16)
            src = x[pair * 2 * H : (pair + 1) * 2 * H, :].rearrange(
                "(i r p) c -> p i r c", p=128, i=2
            )
            nc.gpsimd.dma_start(
                xt[:].rearrange("p (i r c) -> p i r c", i=2, c=W), src
            )

            for half in range(2):
                img = pair * 2 + half
                xa = xt[:, half * 2 * W : (half + 1) * 2 * W]

                # ---- matmul: stationary BD fp16, moving image fp16 ----
                ps = ppool.tile([128, 2 * W], f32)
                nc.tensor.matmul(ps[:], bdt[:], xa, start=True, stop=True)

                # ---- scalar cast PSUM f32 -> SBUF fp16, permuting the free
                # layout to (r, y, jh, jl) = ST1's logical view ----
                zc = zcpool.tile([128, 2 * W], f16)
                nc.scalar.copy(
                    zc[:].rearrange("p (r y jh jl) -> p r jh jl y", r=2, y=8, jh=8, jl=4),
                    ps[:].rearrange("p (r jh jl y) -> p r jh jl y", r=2, jh=8, jl=4, y=8),
                )

                # ---- ST1 fp16 (contiguous in): swap p-low5 (Gl,v) <-> (Jh,Jl);
                # write z1 phys = (r, v, gl, y) = ST2's logical ----
                z1 = z1pool.tile([128, 2 * W], f16)
                nc.vector.transpose(
                    z1[:].rearrange("p (r v gl y) -> p r y gl v", r=2, v=8, gl=4, y=8),
                    zc[:],
                )

                # ---- ST2 fp16 (contiguous in): swap p-low5 (Jh,Jl) <-> (Gl,y);
                # write z2 phys = (r, jh, jl, v) = contiguous store layout ----
                z2 = z2pool.tile([128, 2 * W], f16)
                nc.vector.transpose(
                    z2[:].rearrange("p (r jh jl v) -> p r v jh jl", r=2, jh=8, jl=4, v=8),
                    z1[:],
                )

                # ---- scalar cast back fp16 -> f32 (contiguous) ----
                z3 = z3pool.tile([128, 2 * W], f32)
                nc.scalar.copy(z3[:], z2[:])

                # ---- contiguous store: p = out row, f = (r, col) ----
                dst = o[img * H : (img + 1) * H, :].rearrange(
                    "(r p) c -> p r c", p=128
                )
                nc.sync.dma_start(dst, z3[:].rearrange("p (r c) -> p r c", c=W))


def _build_nc():
    nc = bacc.Bacc(
        "TRN2", target_bir_lowering=False, debug=False, num_devices=NCORES
    )
    x_ap = nc.dram_tensor("x", [ROWS, W], mybir.dt.float16, kind="ExternalInput").ap()
    bd_ap = nc.dram_tensor(
        "bd", [128, 128], mybir.dt.float16, kind="ExternalInput"
    ).ap()
    o_ap = nc.dram_tensor("o", [ROWS, W], mybir.dt.float32, kind="ExternalOutput").ap()
    with tile.TileContext(nc) as tc:
        _dct_kernel(tc, o_ap, x_ap, bd_ap)
    nc.compile()
    return nc


def _make_bd(dct_basis: np.ndarray) -> np.ndarray:
    a = dct_basis.astype(np.float64) @ dct_basis.astype(np.float64)
    at = a.T.astype(np.float32)  # block[x, v] = A[v, x]
    bd = np.zeros((128, 128), dtype=np.float32)
    for g in range(16):
        bd[g * P : (g + 1) * P, g * P : (g + 1) * P] = at
    return bd


def kernel(x: np.ndarray, dct_basis: np.ndarray) -> np.ndarray:
    global _nc_cache, LAST_RESULTS
    x = np.asarray(x, dtype=np.float32)
    dct_basis = np.asarray(dct_basis, dtype=np.float32)
    assert x.shape == (B, C, H, W)

    if _nc_cache is None:
        _nc_cache = _build_nc()
    nc = _nc_cache

    bd = _make_bd(dct_basis).astype(np.float16)
    in_maps = []
    for i in range(NCORES):
        xs = np.ascontiguousarray(
            x[i * BPC : (i + 1) * BPC].reshape(ROWS, W).astype(np.float16)
        )
        in_maps.append({"x": xs, "bd": bd})

    if TRACE:
        _ensure_ntff_hook()
    try:
        res = run_bass_kernel_spmd(
            nc, in_maps, core_ids=list(range(NCORES)), trace=TRACE
        )
    except ModuleNotFoundError:
        res = run_bass_kernel_spmd(
            nc, in_maps, core_ids=list(range(NCORES)), trace=False
        )
    LAST_RESULTS = res

    out = np.empty((B, C, H, W), dtype=np.float32)
    for i in range(NCORES):
        out[i * BPC : (i + 1) * BPC] = res.results[i]["o"].reshape(BPC, C, H, W)
    return out


# revision 12
# speedup vs baseline: 2.3954x; 2.3954x over previous
"""Trainium2 Bass kernel for nn_DCTLayer: per-8x8-block 2D DCT-like transform.

Math: reference computes, per 8x8 block X of the 256x256 image,
    out_block[y, v] = sum_x A[v, x] * X[x, y],   where A = D @ D
(D = 8x8 DCT basis). out_block = (A @ X)^T.

Kernel strategy (per core, pure data parallel over batch):
  - Load 2 images per DMA, casting f32 -> fp16 in the SWDGE DMA (halves
    HBM read bytes). SBUF layout: partition = row (32Gh + 8Gl + x),
    free = (img, r, col) with col = 32Jh + 8Jl + y.
  - ONE matmul per image with the constant 128x128 block-diagonal BD (16
    copies of A^T, fp16) as the STATIONARY operand and the image as the
    512-wide fp16 moving operand (1 cycle/row):
        z[32Gh+8Gl+v, f=(r, 32Jh+8Jl+y)] = sum_x A[v,x] X[row(G,x), col]
  - The required within-8x8-block transpose swaps v (partition-low) with
    y (free-low). Two DVE StreamTranspose passes (32x32 logical block
    transposes) with bit-field relabeling folded into the access
    patterns perform it entirely on-chip:
      ST1: logical f view (r,y | Jh,Jl): swaps p-low5 (Gl,v) <-> (Jh,Jl)
           -> z1[p = 32Gh+4Jh+Jl, f-phys = 256r + 32v+8Gl+y]
      ST2: identity views: swaps (Jh,Jl) <-> (Gl,y)
           -> z2[p = 32Gh+8Gl+y = out row, f-phys = 256r + 32Jh+8Jl+v]
  - Store z2 contiguously in f32 (1KB runs per row).
"""

import sys

sys.path.insert(0, "/opt/trn_rl_repo")

from contextlib import ExitStack

import numpy as np

import concourse.bass as bass  # noqa: F401
import concourse.tile as tile
from concourse import bacc, mybir
from concourse.bass_utils import run_bass_kernel_spmd

P = 8
H = W = 256
B, C = 16, 64
NCORES = 8
BPC = B // NCORES  # batches per core
IMGS = BPC * C  # images (b,c planes) per core
ROWS = IMGS * H  # dram rows per core

TRACE = False
LAST_RESULTS = None

_nc_cache = None


def _ensure_ntff_hook():
    """The agent image's antenv lacks axon_hooks; synthesize it so
    run_bass_kernel_spmd(trace=True) can capture NTFF profiles."""
    import types

    if "antenv.axon_hooks" in sys.modules:
        return
    try:
        sys.path.insert(0, "/root/.axon_site/trn_agent_boot")
        from trn_boot import _ntff_profile_via_ctypes

        hook = _ntff_profile_via_ctypes("/opt/axon/libaxon_pjrt.so")
    except Exception:
        hook = None
    mod = types.ModuleType("antenv.axon_hooks")
    mod._hook = hook
    mod.get_axon_ntff_profile_hook = lambda: mod._hook
    mod.set_axon_ntff_profile_hook = lambda h: setattr(mod, "_hook", h)
    sys.modules["antenv.axon_hooks"] = mod


def _dct_kernel(tc, o, x, bd):
    nc = tc.nc
    f32 = mybir.dt.float32
    f16 = mybir.dt.float16
    with ExitStack() as ctx:
        xpool = ctx.enter_context(tc.tile_pool(name="xin", bufs=3))
        z1pool = ctx.enter_context(tc.tile_pool(name="z1", bufs=4))
        z2pool = ctx.enter_context(tc.tile_pool(name="z2", bufs=4))
        cpool = ctx.enter_context(tc.tile_pool(name="const", bufs=1))
        ppool = ctx.enter_context(tc.tile_pool(name="ps", bufs=8, space="PSUM"))

        bdt = cpool.tile([128, 128], f16)
        nc.gpsimd.dma_start(bdt[:], bd[:])

        for pair in range(IMGS // 2):
            # ---- load 2 fp16 images as [p=row%128, (i r c)], 512B runs ----
            xt = xpool.tile([128, 4 * W], f16)
            src = x[pair * 2 * H : (pair + 1) * 2 * H, :].rearrange(
                "(i r p) c -> p i r c", p=128, i=2
            )
            nc.gpsimd.dma_start(
                xt[:].rearrange("p (i r c) -> p i r c", i=2, c=W), src
            )

            for half in range(2):
                img = pair * 2 + half
                xa = xt[:, half * 2 * W : (half + 1) * 2 * W]

                # ---- matmul: stationary BD fp16, moving image fp16 ----
                ps = ppool.tile([128, 2 * W], f32)
                nc.tensor.matmul(ps[:], bdt[:], xa, start=True, stop=True)

                # ---- ST1 f32: swap p-low5 (Gl,v) <-> f-low5 (Jh,Jl) ----
                z1 = z1pool.tile([128, 2 * W], f32)
                nc.vector.transpose(
                    z1[:].rearrange("p (r v gl y) -> p r y gl v", r=2, v=8, gl=4, y=8),
                    ps[:].rearrange("p (r jh jl y) -> p r y jh jl", r=2, jh=8, jl=4, y=8),
                )

                # ---- ST2 f32: swap p-low5 (Jh,Jl) <-> f-low5 (Gl,y) ----
                z2 = z2pool.tile([128, 2 * W], f32)
                nc.vector.transpose(
                    z2[:].rearrange("p (r jh jl v) -> p r v jh jl", r=2, jh=8, jl=4, v=8),
                    z1[:],
                )

                # ---- contiguous store: p = out row, f = (r, col) ----
                dst = o[img * H : (img + 1) * H, :].rearrange(
                    "(r p) c -> p r c", p=128
                )
                eng = nc.sync if img % 2 == 0 else nc.scalar
                eng.dma_start(dst, z2[:].rearrange("p (r c) -> p r c", c=W))


def _build_nc():
    nc = bacc.Bacc(
        "TRN2", target_bir_lowering=False, debug=False, num_devices=NCORES
    )
    x_ap = nc.dram_tensor("x", [ROWS, W], mybir.dt.float16, kind="ExternalInput").ap()
    bd_ap = nc.dram_tensor(
        "bd", [128, 128], mybir.dt.float16, kind="ExternalInput"
    ).ap()
    o_ap = nc.dram_tensor("o", [ROWS, W], mybir.dt.float32, kind="ExternalOutput").ap()
    with tile.TileContext(nc) as tc:
        _dct_kernel(tc, o_ap, x_ap, bd_ap)
    nc.compile()
    return nc


def _make_bd(dct_basis: np.ndarray) -> np.ndarray:
    a = dct_basis.astype(np.float64) @ dct_basis.astype(np.float64)
    at = a.T.astype(np.float32)  # block[x, v] = A[v, x]
    bd = np.zeros((128, 128), dtype=np.float32)
    for g in range(16):
        bd[g * P : (g + 1) * P, g * P : (g + 1) * P] = at
    return bd


def kernel(x: np.ndarray, dct_basis: np.ndarray) -> np.ndarray:
    global _nc_cache, LAST_RESULTS
    x = np.asarray(x, dtype=np.float32)
    dct_basis = np.asarray(dct_basis, dtype=np.float32)
    assert x.shape == (B, C, H, W)

    if _nc_cache is None:
        _nc_cache = _build_nc()
    nc = _nc_cache

    bd = _make_bd(dct_basis).astype(np.float16)
    in_maps = []
    for i in range(NCORES):
        xs = np.ascontiguousarray(
            x[i * BPC : (i + 1) * BPC].reshape(ROWS, W).astype(np.float16)
        )
        in_maps.append({"x": xs, "bd": bd})

    if TRACE:
        _ensure_ntff_hook()
    try:
        res = run_bass_kernel_spmd(
            nc, in_maps, core_ids=list(range(NCORES)), trace=TRACE
        )
    except ModuleNotFoundError:
        res = run_bass_kernel_spmd(
            nc, in_maps, core_ids=list(range(NCORES)), trace=False
        )
    LAST_RESULTS = res

    out = np.empty((B, C, H, W), dtype=np.float32)
    for i in range(NCORES):
        out[i * BPC : (i + 1) * BPC] = res.results[i]["o"].reshape(BPC, C, H, W)
    return out
